# revision 12
# baseline (speedup 1.0000x reference)
"""Multi-head self-attention (RoPE, causal) TRN2 Bass kernel.

Problem: B=4, S=2048, D=1024, H=16, Dh=64, fp32 in/out.

Sharding (8 cores): DP=2 over batch pairs x TP=4 over heads
(Megatron-style).  Core c = bg*4 + tp handles batches {2bg, 2bg+1} with
heads tp*4 .. tp*4+3 and produces partial outputs [2, D, S]; the host
sums the four TP partials per batch (the all-reduce after
out_projection).  Vs. DP=4xTP=2 this halves the per-core QKV-projection
FLOPs (the largest single TensorE block) at the cost of a 2x larger
input/output DMA footprint, which overlaps with compute.

Single fused device pipeline (matmul operands bf16, PSUM fp32):
  QKV block k+2 / attention row k / out-projection rows are emitted
  interleaved so TensorE never drains (HAM stays warm) and ScalarE's
  exp stream is fed continuously.  Rows/blocks ordered
  (b0,0),(b1,0),(b0,1),...  PSUM (8 banks): qkv-accum 2 | scores 2x2 |
  ctx 2, with the qkv pool released before the out-proj pool opens.

Per-core program details:
  QKV: Q^T/K^T in [e, t] layout (RoPE pair components pre-permuted via W
    row perm), V natural [t, dv] + ones column (softmax denominator
    rides row 64 of the ctx accumulation).  RoPE: partition-swap via
    SBUF DMA (gpsimd queue) + 3 DVE bf16 ops.  Early-block PSUM
    evictions run on ScalarE (idle until exp volume ramps), later ones
    on VectorE.
  Attention: transposed scores S^T = K_blk^T.T @ Q^T, head pairs
    row-packed on the PE (rows 0-63 / 64-127), one ACT exp per
    (b,i,p,j)-unit -> bf16 P, triangular mask on diagonal blocks,
    ctx[65, q] += V_aug.T @ P^T.  AV matmuls trail the score matmuls by
    one unit (software pipeline) so the in-order PE never waits on exp.
  Norm: ctx rows 0-64 stashed bf16 (even head straight into qt's dead
    columns), bf16 denominator rows gathered to [4, 512] (sync queue),
    one reciprocal, per-pair one-hot matmul broadcast, one [128, 512]
    multiply per pair.
  Out-proj: outT[e, t] += wo_p.T @ ctx_pair over 2 pairs, fp32 out.
"""

import sys

for _p in ("/opt/trn_rl_repo", "/root/.axon_site/_ro/trn_rl_repo"):
    if _p not in sys.path:
        sys.path.insert(0, _p)

import ml_dtypes
import numpy as np

import concourse.bacc as bacc
import concourse.mybir as mybir
import concourse.tile as tile
from concourse.bass_utils import run_bass_kernel_spmd

F32 = mybir.dt.float32
F32R = mybir.dt.float32r
BF16 = mybir.dt.bfloat16
EXP = mybir.ActivationFunctionType.Exp

B, S, D = 4, 2048, 1024
H, DH = 16, 64
THETA = 10000.0
NCORES = 8
DP, TP = 2, 4                        # batch-pair groups x head groups
NB = B // DP                         # 2 batches per core
HLOC = H // TP                       # 4 local heads
NPAIR = HLOC // 2                    # 2 head pairs
NT = S // 512                        # 4 q/t tiles of 512
NTQ = S // 128                       # 16 k-chunks of 128
ND = D // 128                        # 8 d-chunks
NQK = 2 * NPAIR                      # 4 q/k e-chunks per (b, ts)
SCALE = 1.0 / 8.0                    # 1/sqrt(DH)

_PROGRAM = None


def _build_program():
    nc = bacc.Bacc(None)

    x_d = nc.dram_tensor("x", [128, NB, NT, ND, 512], BF16,
                         kind="ExternalInput")
    wqkv_d = nc.dram_tensor("wqkv", [ND, 128, 3 * HLOC * DH], BF16,
                            kind="ExternalInput")
    wo_d = nc.dram_tensor("wo", [128, NPAIR, D], BF16, kind="ExternalInput")
    cos_d = nc.dram_tensor("cosT", [128, S], BF16, kind="ExternalInput")
    sin_d = nc.dram_tensor("sinT", [128, S], BF16, kind="ExternalInput")
    mask_d = nc.dram_tensor("mask", [128, 128], BF16, kind="ExternalInput")
    onehot_d = nc.dram_tensor("onehot", [HLOC, NPAIR * 128], F32R,
                              kind="ExternalInput")
    out_d = nc.dram_tensor("out", [NB, D, S], F32, kind="ExternalOutput")

    # interleave order for QKV blocks and attention rows
    ORDER = [(b, t) for t in range(NT) for b in range(NB)]

    with tile.TileContext(nc) as tc:
        with (
            tc.tile_pool(name="const", bufs=1) as constp,
            tc.tile_pool(name="xw", bufs=1) as xwp,
            tc.tile_pool(name="qkv", bufs=1) as qkp,
            tc.tile_pool(name="vpool", bufs=1) as vpool,
            tc.tile_pool(name="rope", bufs=1) as ropep,
            tc.tile_pool(name="attn", bufs=1) as attnp,
            tc.tile_pool(name="norm", bufs=1) as normp,
            tc.tile_pool(name="proj", bufs=1) as projp,
            tc.tile_pool(name="stp", bufs=1, space="PSUM") as stp,
            tc.tile_pool(name="cdp", bufs=1, space="PSUM") as cdp,
        ):
            # ---- input DMAs spread over 4 queues; first-block deps first --
            w_sb = [xwp.tile([128, 3 * HLOC * DH], BF16, name=f"w{d}")
                    for d in range(ND)]
            xa = xwp.tile([128, NB, NT, ND, 512], BF16, name="xa")
            cos_sb = constp.tile([128, S], BF16)
            sin_sb = constp.tile([128, S], BF16)
            for d in range(ND):
                (nc.sync if d % 2 == 0 else nc.scalar).dma_start(
                    w_sb[d][:], wqkv_d[d])
            nc.scalar.dma_start(xa[:, 0, 0], x_d[:, 0, 0])
            nc.gpsimd.dma_start(xa[:, 1, 0], x_d[:, 1, 0])
            nc.sync.dma_start(cos_sb[:], cos_d[:])
            nc.sync.dma_start(sin_sb[:], sin_d[:])
            for ts in range(1, NT):
                nc.scalar.dma_start(xa[:, 0, ts], x_d[:, 0, ts])
                nc.gpsimd.dma_start(xa[:, 1, ts], x_d[:, 1, ts])
            mask_sb = constp.tile([128, 128], BF16)
            nc.sync.dma_start(mask_sb[:], mask_d[:])
            onehot_sb = constp.tile([HLOC, NPAIR * 128], F32R)
            nc.sync.dma_start(onehot_sb[:], onehot_d[:])
            wo_sb = constp.tile([128, NPAIR, D], BF16)
            nc.sync.dma_start(wo_sb[:], wo_d[:])

            qt = {(b, p): qkp.tile([128, S], BF16, name=f"qt{b}{p}")
                  for b in range(NB) for p in range(NPAIR)}
            kt = {(b, p): qkp.tile([128, S], BF16, name=f"kt{b}{p}")
                  for b in range(NB) for p in range(NPAIR)}
            vt = {(b, t): vpool.tile([128, HLOC, DH + 1], BF16,
                                     name=f"v{b}_{t}")
                  for b in range(NB) for t in range(NTQ)}
            for v in vt.values():
                nc.vector.memset(v[:, :, DH:DH + 1], 1.0)

            qkv_pool = tc.alloc_tile_pool(name="ps1", bufs=1, space="PSUM")

            # ---------------- QKV projection + RoPE ----------------
            def emit_qkv_chain(blk, b, ts, c):
                # early blocks evict on ScalarE (idle); later on VectorE
                evict = (nc.scalar.copy if blk < 4
                         else nc.vector.tensor_copy)
                tsl = slice(ts * 512, (ts + 1) * 512)
                if c < NQK:             # Q (c<NPAIR) or K e-chunk
                    e = c
                    ps = qkv_pool.tile([128, 512], F32, tag="qkps", bufs=2,
                                       name=f"ps_{b}_{ts}_{c}")
                    for d in range(ND):
                        nc.tensor.matmul(
                            ps[:], w_sb[d][:, e * 128:(e + 1) * 128],
                            xa[:, b, ts, d, :],
                            start=(d == 0), stop=(d == ND - 1),
                        )
                    dst = qt[(b, e)] if e < NPAIR else kt[(b, e - NPAIR)]
                    evict(dst[:, tsl], ps[:])
                    # RoPE in place: quadrant swap via SBUF->SBUF DMA
                    sw = ropep.tile([128, 512], BF16, tag="sw", bufs=2)
                    for qd in range(4):
                        sq = qd ^ 1
                        nc.gpsimd.dma_start(
                            sw[qd * 32:(qd + 1) * 32, :],
                            dst[sq * 32:(sq + 1) * 32, tsl],
                        )
                    t1 = ropep.tile([128, 512], BF16, tag="t1", bufs=2)
                    nc.vector.tensor_mul(t1[:], dst[:, tsl], cos_sb[:, tsl])
                    nc.vector.tensor_mul(sw[:], sw[:], sin_sb[:, tsl])
                    nc.vector.tensor_add(dst[:, tsl], t1[:], sw[:])
                else:                   # V chunk (natural [t, dv] layout)
                    tq0 = c - NQK
                    tq = ts * 4 + tq0
                    psv = qkv_pool.tile([128, HLOC * DH], F32, tag="qkps",
                                        bufs=2, name=f"psv_{b}_{ts}_{tq0}")
                    for d in range(ND):
                        nc.tensor.matmul(
                            psv[:],
                            xa[:, b, ts, d, tq0 * 128:(tq0 + 1) * 128],
                            w_sb[d][:, 2 * HLOC * DH:3 * HLOC * DH],
                            start=(d == 0), stop=(d == ND - 1),
                        )
                    evict(
                        vt[(b, tq)][:, :, 0:DH],
                        psv.rearrange("p (h d) -> p h d", h=HLOC),
                    )

            # ---------------- attention ----------------
            ctx_ps = {}

            def emit_scores(b, i, p, j):
                lo = max(0, 128 * j - 512 * i)
                qsl = slice(512 * i + lo, 512 * (i + 1))
                ksl = slice(j * 128, (j + 1) * 128)
                st = stp.tile([128, 2, 512], F32, tag="st", bufs=2,
                              name=f"st_{b}_{i}_{p}_{j}")
                nc.tensor.matmul(
                    st[:, 0, lo:512], kt[(b, p)][0:64, ksl],
                    qt[(b, p)][0:64, qsl], tile_position=(0, 0),
                )
                nc.tensor.matmul(
                    st[:, 1, lo:512], kt[(b, p)][64:128, ksl],
                    qt[(b, p)][64:128, qsl], tile_position=(64, 0),
                )
                pt = attnp.tile([128, 2, 512], BF16, tag="pt", bufs=4,
                                name=f"pt_{b}_{i}_{p}_{j}")
                nc.scalar.activation(
                    pt[:, :, lo:512], st[:, :, lo:512], EXP, scale=SCALE,
                )
                if lo == 128 * j - 512 * i:  # block starts on the diagonal
                    nc.vector.tensor_mul(
                        pt[:, :, lo:lo + 128],
                        pt[:, :, lo:lo + 128],
                        mask_sb[:, None, :].to_broadcast([128, 2, 128]),
                    )
                return pt

            def emit_av(b, i, p, j, nj, pt):
                lo = max(0, 128 * j - 512 * i)
                if j == 0:
                    ctx_ps[(p, 0)] = cdp.tile([65, 512], F32, tag="ctxA",
                                              bufs=1, name=f"cA_{b}_{i}_{p}")
                    ctx_ps[(p, 1)] = cdp.tile([65, 512], F32, tag="ctxB",
                                              bufs=1, name=f"cB_{b}_{i}_{p}")
                nc.tensor.matmul(
                    ctx_ps[(p, 0)][:, lo:512], vt[(b, j)][:, 2 * p, :],
                    pt[:, 0, lo:512],
                    start=(j == 0), stop=(j == nj - 1),
                )
                nc.tensor.matmul(
                    ctx_ps[(p, 1)][:, lo:512], vt[(b, j)][:, 2 * p + 1, :],
                    pt[:, 1, lo:512],
                    start=(j == 0), stop=(j == nj - 1),
                )

            def emit_stash(b, i, p, den_g):
                """After last AV of (b, i, p): evict ctx (bf16, row 64 =
                denominator), gather den rows into den_g (sync queue)."""
                tsl = slice(512 * i, (i + 1) * 512)
                cA, cB = ctx_ps[(p, 0)], ctx_ps[(p, 1)]
                # even head (rows 0-64 incl den row) into qt dead columns;
                # row 64 clobbers odd head's first Q row - already consumed
                nc.vector.tensor_copy(qt[(b, p)][0:65, tsl], cA[:, :])
                stashB = normp.tile([65, 512], BF16, tag="stB", bufs=2,
                                    name=f"stB_{b}_{i}_{p}")
                nc.vector.tensor_copy(stashB[:, :], cB[:, :])
                nc.sync.dma_start(den_g[2 * p:2 * p + 1, :],
                                  qt[(b, p)][64:65, tsl])
                nc.sync.dma_start(den_g[2 * p + 1:2 * p + 2, :],
                                  stashB[64:65, :])
                # odd head: repack into qt rows 64-127 (partition move)
                nc.sync.dma_start(qt[(b, p)][64:128, tsl], stashB[0:64, :])

            def emit_norm_fillers(b, i, den_g):
                tsl = slice(512 * i, (i + 1) * 512)
                rec = normp.tile([HLOC, 512], F32, tag="rec", bufs=2,
                                 name=f"rec_{b}_{i}")
                den_f = normp.tile([HLOC, 512], F32, tag="denf", bufs=2,
                                   name=f"denf_{b}_{i}")
                rec_r = normp.tile([HLOC, 512], F32R, tag="recr", bufs=2,
                                   name=f"recr_{b}_{i}")

                def recip():
                    nc.vector.tensor_copy(den_f[:], den_g[:])
                    nc.vector.reciprocal_approx_fast(rec[:], den_f[:])
                    nc.vector.tensor_copy(rec_r[:], rec[:])

                def norm_pair(p):
                    bc = stp.tile([128, 512], F32, tag="st", bufs=2,
                                  name=f"bc_{b}_{i}_{p}")
                    nc.tensor.matmul(
                        bc[:], onehot_sb[:, p * 128:(p + 1) * 128], rec_r[:])
                    bc_sb = normp.tile([128, 512], F32, tag="bcsb", bufs=2,
                                       name=f"bcsb_{b}_{i}_{p}")
                    nc.vector.tensor_copy(bc_sb[:], bc[:])
                    nc.vector.tensor_mul(
                        qt[(b, p)][:, tsl], qt[(b, p)][:, tsl], bc_sb[:])

                return [recip] + [
                    (lambda p_: (lambda: norm_pair(p_)))(p)
                    for p in range(NPAIR)]

            # ---------------- out projection ----------------
            pso_pool = [None]  # opened after qkv_pool releases

            def emit_proj_chunk(b, i, ec):
                tsl = slice(i * 512, (i + 1) * 512)
                ecs = slice(ec * 128, (ec + 1) * 128)
                pso = pso_pool[0].tile([128, 512], F32, tag="pso", bufs=2,
                                       name=f"pso_{b}_{i}_{ec}")
                for p in range(NPAIR):
                    nc.tensor.matmul(
                        pso[:], wo_sb[:, p, ecs], qt[(b, p)][:, tsl],
                        start=(p == 0), stop=(p == NPAIR - 1),
                    )
                ot = projp.tile([128, 512], F32, tag="ot", bufs=4,
                                name=f"ot_{b}_{i}_{ec}")
                nc.vector.tensor_copy(ot[:], pso[:])
                nc.sync.dma_start(out_d[b, ecs, tsl], ot[:])

            def proj_fillers(rows):
                return [(lambda b, i, ec: (lambda: emit_proj_chunk(b, i, ec)))
                        (b, i, ec) for (b, i) in rows for ec in range(ND)]

            # ---------------- emission schedule ----------------
            den_gs = {}

            def emit_attn_row(b, i, fillers):
                """Row (b, i) with fillers interleaved; AV matmuls trail the
                score matmuls by one unit (software pipeline)."""
                nj = 4 * i + 4
                units = [(p, j) for p in range(NPAIR) for j in range(nj)]
                den_g = normp.tile([HLOC, 512], BF16, tag="deng", bufs=2,
                                   name=f"deng_{b}_{i}")
                den_gs[(b, i)] = den_g
                nf, nu = len(fillers), len(units)
                fi = 0
                pending = None
                for u, (p, j) in enumerate(units):
                    while fi < nf and fi * nu <= u * nf:
                        fillers[fi]()
                        fi += 1
                    pt = emit_scores(b, i, p, j)
                    if pending is not None:
                        emit_av(*pending)
                        if pending[3] == nj - 1:  # last j of previous pair
                            emit_stash(b, i, pending[2], den_g)
                    pending = (b, i, p, j, nj, pt)
                emit_av(*pending)
                emit_stash(b, i, NPAIR - 1, den_g)
                while fi < nf:
                    fillers[fi]()
                    fi += 1
                return emit_norm_fillers(b, i, den_g)

            # two QKV blocks up front
            for blk in range(2):
                b, ts = ORDER[blk]
                for c in range(NQK + 4):
                    emit_qkv_chain(blk, b, ts, c)
            # stages 0..5: attention row k with QKV block k+2 as filler
            norm_f = []
            for k in range(6):
                b, i = ORDER[k]
                blk = k + 2
                bq, tq_ = ORDER[blk]
                chains = [
                    (lambda a, bb, t, c: (lambda: emit_qkv_chain(a, bb, t, c)))
                    (blk, bq, tq_, c) for c in range(NQK + 4)]
                norm_f = emit_attn_row(b, i, norm_f + chains)
            # qkv accum banks -> out-projection accum banks
            qkv_pool.release()
            pso_pool[0] = tc.alloc_tile_pool(name="psop", bufs=1, space="PSUM")
            # stage 6: row (b0,3); fillers: norm(b1,2) + proj rows 0..2
            b, i = ORDER[6]
            norm_f = emit_attn_row(b, i, norm_f + proj_fillers(ORDER[0:3]))
            # stage 7: row (b1,3); fillers: norm(b0,3) + proj rows 3..5,
            # then proj row 6 at the end (after norm(b0,3) completes)
            b, i = ORDER[7]
            norm_f = emit_attn_row(
                b, i,
                norm_f + proj_fillers(ORDER[3:6]) + proj_fillers(ORDER[6:7]))
            # tail: norm(b1,3) + proj row 7
            for f in norm_f:
                f()
            for ec in range(ND):
                emit_proj_chunk(ORDER[7][0], ORDER[7][1], ec)
            pso_pool[0].release()

    nc.compile()
    return nc


def _get_program():
    global _PROGRAM
    if _PROGRAM is None:
        _PROGRAM = _build_program()
    return _PROGRAM


def _bf16(a):
    return np.ascontiguousarray(a.astype(ml_dtypes.bfloat16))


def _prep_in_maps(in_features, token_positions, W_qkv, W_out):
    in_features = np.asarray(in_features, dtype=np.float32)
    token_positions = np.asarray(token_positions)
    W_qkv = np.asarray(W_qkv, dtype=np.float32)
    W_out = np.asarray(W_out, dtype=np.float32)

    # RoPE pair permutation: [x0 of freq 0..31 | x1 of freq 0..31]
    perm = np.concatenate([np.arange(0, DH, 2), np.arange(1, DH, 2)])

    wqkv, wo = [], []
    for tp in range(TP):
        rows = []
        for sect in range(2):  # Q, K (permuted)
            for h in range(HLOC):
                g = tp * HLOC + h
                rows.append(W_qkv[sect * D + g * DH + perm])
        for h in range(HLOC):  # V natural
            g = tp * HLOC + h
            rows.append(W_qkv[2 * D + g * DH:2 * D + (g + 1) * DH])
        Wl = np.concatenate(rows, axis=0)      # [3*HLOC*DH, 1024]
        wqkv.append(_bf16(Wl.T.reshape(ND, 128, 3 * HLOC * DH)))
        # wo[r, p, e] = W_out[e, (tp*HLOC + 2p + r//64)*DH + r%64]
        w = np.stack([
            np.concatenate([
                W_out[:, (tp * HLOC + 2 * p) * DH:
                      (tp * HLOC + 2 * p + 1) * DH].T,
                W_out[:, (tp * HLOC + 2 * p + 1) * DH:
                      (tp * HLOC + 2 * p + 2) * DH].T,
            ], axis=0) for p in range(NPAIR)
        ])                                     # [p, 128, 1024]
        wo.append(_bf16(w.transpose(1, 0, 2)))

    half = DH // 2
    inv_freq = (THETA ** (-2.0 * np.arange(half, dtype=np.float32) / DH)
                ).astype(np.float32)
    ang = token_positions.astype(np.float32)[:, None] * inv_freq[None, :]
    cos_t = np.cos(ang).T.astype(np.float32)   # [32, S]
    sin_t = np.sin(ang).T.astype(np.float32)
    cos128 = _bf16(np.tile(cos_t, (4, 1)))
    sin128 = _bf16(np.tile(np.concatenate([-sin_t, sin_t], axis=0), (2, 1)))
    # mask[kv, c] = 1 iff kv <= c (scores stored transposed: [kv, q])
    mask128 = _bf16(np.triu(np.ones((128, 128), dtype=np.float32)))
    # onehot[h, p*128 + c] = 1 iff h == 2p + c//64
    onehot = np.zeros((HLOC, NPAIR * 128), dtype=np.float32)
    for p in range(NPAIR):
        onehot[2 * p, p * 128:p * 128 + 64] = 1.0
        onehot[2 * p + 1, p * 128 + 64:p * 128 + 128] = 1.0

    in_maps = []
    for c in range(NCORES):
        bg, tp = c // TP, c % TP
        xh = _bf16(np.stack([
            in_features[2 * bg + b].reshape(NT, 512, ND, 128)
            .transpose(3, 0, 2, 1) for b in range(NB)
        ], axis=1))                            # [128, NB, NT, ND, 512]
        in_maps.append({
            "x": xh,
            "wqkv": wqkv[tp],
            "wo": wo[tp],
            "cosT": cos128,
            "sinT": sin128,
            "mask": mask128,
            "onehot": onehot,
        })
    return in_maps


def run(in_features, token_positions, W_qkv, W_out, **spmd_kwargs):
    """Run the kernel; returns (output [B,S,D] f32, BassKernelResults)."""
    in_maps = _prep_in_maps(in_features, token_positions, W_qkv, W_out)
    nc = _get_program()
    res = run_bass_kernel_spmd(nc, in_maps, core_ids=list(range(NCORES)),
                               **spmd_kwargs)
    full = np.stack([
        sum(res.results[(b // NB) * TP + tp]["out"][b % NB]
            for tp in range(TP)).T
        for b in range(B)
    ])
    return full.astype(np.float32), res


def kernel(in_features, token_positions, W_qkv, W_out):
    out, _ = run(in_features, token_positions, W_qkv, W_out)
    return out


# revision 20
# speedup vs baseline: 1.0272x; 1.0272x over previous
"""Multi-head self-attention (RoPE, causal) TRN2 Bass kernel.

Problem: B=4, S=2048, D=1024, H=16, Dh=64, fp32 in/out.

Sharding (8 cores): DP=4 over batch x TP=2 over heads (Megatron-style).
Core c handles batch c//2 with heads (c%2)*8 .. (c%2)*8+7 and produces a
partial output [D, S]; the host sums the two TP partials per batch (the
all-reduce after out_projection).

Single fused device pipeline (all matmul operands bf16, PSUM fp32):
  QKV tile ts / attention row ts-1 / out-projection are emitted
  interleaved so TensorE never drains (HAM stays warm) and ScalarE's exp
  stream starts as soon as the first score tile exists.
    stage 0:  QKV(0) + RoPE(0)
    stage s:  QKV(s) chains interleaved with attention row s-1 units
    stage 4:  attention row 3 interleaved with out-proj rows 0..2
  PSUM budget (8 banks): qkv-accum 2 | scores 2x2 | ctx 2, with the qkv
  pool released before the out-proj pool opens (proj reuses its banks).

Per-core program details:
  QKV: Q^T/K^T in [e, t] layout (RoPE pair components pre-permuted via W
    row perm), V natural [t, dv] + ones column (softmax denominator
    rides row 64 of the ctx accumulation).  RoPE: partition-swap via
    SBUF DMA (gpsimd queue) + 3 DVE ops on bf16 (2x rate).
  Attention: transposed scores S^T = K_blk^T.T @ Q^T, head pairs
    row-packed on the PE (rows 0-63 / 64-127), one ACT exp per
    (i,p,j)-unit -> bf16 P, triangular mask on diagonal blocks,
    ctx[65, q] += V_aug.T @ P^T.
  Norm: ctx rows 0-63 stashed bf16 (even head straight into qt's dead
    columns), denominator row kept fp32, gathered to [8, 512], one
    reciprocal, per-pair one-hot matmul broadcast, one [128, 512]
    multiply per pair.
  Out-proj: outT[e, t] += wo_p.T @ ctx_pair over 4 pairs, fp32 out.
"""

import sys

for _p in ("/opt/trn_rl_repo", "/root/.axon_site/_ro/trn_rl_repo"):
    if _p not in sys.path:
        sys.path.insert(0, _p)

import ml_dtypes
import numpy as np

import concourse.bacc as bacc
import concourse.bass_utils as bass_utils
import concourse.mybir as mybir
import concourse.tile as tile
from concourse.bass_utils import run_bass_kernel_spmd

F32 = mybir.dt.float32
F32R = mybir.dt.float32r
BF16 = mybir.dt.bfloat16
EXP = mybir.ActivationFunctionType.Exp

B, S, D = 4, 2048, 1024
H, DH = 16, 64
THETA = 10000.0
NCORES, TP, HLOC = 8, 2, 8
NPAIR = HLOC // 2
NT = S // 512                        # 4 q/t tiles of 512
NTQ = S // 128                       # 16 k-chunks of 128
ND = D // 128                        # 8 d-chunks
SCALE = 1.0 / 8.0                    # 1/sqrt(DH)

_PROGRAM = None


def _build_program():
    nc = bacc.Bacc(None)

    x_d = nc.dram_tensor("x", [128, NT, ND, 512], BF16, kind="ExternalInput")
    wqkv_d = nc.dram_tensor("wqkv", [ND, 128, 3 * HLOC * DH], BF16,
                            kind="ExternalInput")
    wo_d = nc.dram_tensor("wo", [128, NPAIR, D], BF16, kind="ExternalInput")
    cos_d = nc.dram_tensor("cosT", [128, S], BF16, kind="ExternalInput")
    sin_d = nc.dram_tensor("sinT", [128, S], BF16, kind="ExternalInput")
    mask_d = nc.dram_tensor("mask", [128, 128], BF16, kind="ExternalInput")
    onehot_d = nc.dram_tensor("onehot", [8, NPAIR * 128], F32R,
                              kind="ExternalInput")
    out_d = nc.dram_tensor("out", [D, S], F32, kind="ExternalOutput")

    with tile.TileContext(nc) as tc:
        with (
            tc.tile_pool(name="const", bufs=1) as constp,
            tc.tile_pool(name="xw", bufs=1) as xwp,
            tc.tile_pool(name="qkv", bufs=1) as qkp,
            tc.tile_pool(name="vpool", bufs=1) as vpool,
            tc.tile_pool(name="rope", bufs=1) as ropep,
            tc.tile_pool(name="attn", bufs=1) as attnp,
            tc.tile_pool(name="norm", bufs=1) as normp,
            tc.tile_pool(name="proj", bufs=1) as projp,
            tc.tile_pool(name="stp", bufs=1, space="PSUM") as stp,
            tc.tile_pool(name="cdp", bufs=1, space="PSUM") as cdp,
        ):
            # ---- input DMAs: W split by Q/K/V section so the first score
            # chains' dependencies (Q sections + x tile 0) land first;
            # spread over the sync and scalar queues (gpsimd stays free
            # for the RoPE partition-swap DMAs).
            w_sb = [xwp.tile([128, 3 * HLOC * DH], BF16, name=f"w{d}")
                    for d in range(ND)]
            xa = xwp.tile([128, NT, ND, 512], BF16, name="xa")
            sect = HLOC * DH
            for d in range(ND):
                nc.sync.dma_start(w_sb[d][:, 0:sect], wqkv_d[d, :, 0:sect])
            nc.scalar.dma_start(xa[:, 0], x_d[:, 0])
            for d in range(ND):
                nc.sync.dma_start(w_sb[d][:, sect:2 * sect],
                                  wqkv_d[d, :, sect:2 * sect])
            cos_sb = constp.tile([128, S], BF16)
            sin_sb = constp.tile([128, S], BF16)
            nc.scalar.dma_start(cos_sb[:], cos_d[:])
            nc.scalar.dma_start(sin_sb[:], sin_d[:])
            for d in range(ND):
                nc.sync.dma_start(w_sb[d][:, 2 * sect:3 * sect],
                                  wqkv_d[d, :, 2 * sect:3 * sect])
            for ts in range(1, NT):
                nc.scalar.dma_start(xa[:, ts], x_d[:, ts])
            mask_sb = constp.tile([128, 128], BF16)
            nc.sync.dma_start(mask_sb[:], mask_d[:])
            onehot_sb = constp.tile([8, NPAIR * 128], F32R)
            nc.sync.dma_start(onehot_sb[:], onehot_d[:])
            wo_sb = constp.tile([128, NPAIR, D], BF16)
            nc.sync.dma_start(wo_sb[:], wo_d[:])

            qt = [qkp.tile([128, S], BF16, name=f"qt{p}") for p in range(NPAIR)]
            kt = [qkp.tile([128, S], BF16, name=f"kt{p}") for p in range(NPAIR)]
            vt = [vpool.tile([128, HLOC, DH + 1], BF16, name=f"v{t}")
                  for t in range(NTQ)]
            for t in range(NTQ):
                nc.vector.memset(vt[t][:, :, DH:DH + 1], 1.0)

            qkv_pool = tc.alloc_tile_pool(name="ps1", bufs=1, space="PSUM")

            # ---------------- QKV projection + RoPE ----------------
            def emit_qkv_chain(ts, c):
                # early tiles evict on ScalarE (idle until exp ramps)
                evict = (nc.scalar.copy if ts < 2 else nc.vector.tensor_copy)
                tsl = slice(ts * 512, (ts + 1) * 512)
                if c < 2 * NPAIR:       # Q (c<4) or K (c>=4) e-chunk
                    e = c
                    ps = qkv_pool.tile([128, 512], F32, tag="qkps", bufs=2,
                                       name=f"ps_{ts}_{c}")
                    for d in range(ND):
                        nc.tensor.matmul(
                            ps[:], w_sb[d][:, e * 128:(e + 1) * 128],
                            xa[:, ts, d, :],
                            start=(d == 0), stop=(d == ND - 1),
                        )
                    dst = qt[e] if e < NPAIR else kt[e - NPAIR]
                    evict(dst[:, tsl], ps[:])
                    # RoPE in place: quadrant swap via SBUF->SBUF DMA
                    sw = ropep.tile([128, 512], BF16, tag="sw", bufs=2)
                    for qd in range(4):
                        sq = qd ^ 1
                        nc.gpsimd.dma_start(
                            sw[qd * 32:(qd + 1) * 32, :],
                            dst[sq * 32:(sq + 1) * 32, tsl],
                        )
                    t1 = ropep.tile([128, 512], BF16, tag="t1", bufs=2)
                    nc.vector.tensor_mul(t1[:], dst[:, tsl], cos_sb[:, tsl])
                    nc.vector.tensor_mul(sw[:], sw[:], sin_sb[:, tsl])
                    nc.vector.tensor_add(dst[:, tsl], t1[:], sw[:])
                else:                   # V chunk (natural [t, dv] layout)
                    tq0 = c - 2 * NPAIR
                    tq = ts * 4 + tq0
                    psv = qkv_pool.tile([128, 512], F32, tag="qkps", bufs=2,
                                        name=f"psv_{ts}_{tq0}")
                    for d in range(ND):
                        nc.tensor.matmul(
                            psv[:],
                            xa[:, ts, d, tq0 * 128:(tq0 + 1) * 128],
                            w_sb[d][:, 2 * HLOC * DH:3 * HLOC * DH],
                            start=(d == 0), stop=(d == ND - 1),
                        )
                    evict(
                        vt[tq][:, :, 0:DH],
                        psv.rearrange("p (h d) -> p h d", h=HLOC),
                    )

            # ---------------- attention ----------------
            ctx_ps = {}

            def emit_scores(i, p, j):
                """Score pair-matmul + exp -> pt (bf16).  Returns pt."""
                lo = max(0, 128 * j - 512 * i)
                qsl = slice(512 * i + lo, 512 * (i + 1))
                ksl = slice(j * 128, (j + 1) * 128)
                st = stp.tile([128, 2, 512], F32, tag="st", bufs=2,
                              name=f"st_{i}_{p}_{j}")
                nc.tensor.matmul(
                    st[:, 0, lo:512], kt[p][0:64, ksl],
                    qt[p][0:64, qsl], tile_position=(0, 0),
                )
                nc.tensor.matmul(
                    st[:, 1, lo:512], kt[p][64:128, ksl],
                    qt[p][64:128, qsl], tile_position=(64, 0),
                )
                pt = attnp.tile([128, 2, 512], BF16, tag="pt", bufs=4,
                                name=f"pt_{i}_{p}_{j}")
                nc.scalar.activation(
                    pt[:, :, lo:512], st[:, :, lo:512], EXP, scale=SCALE,
                )
                if lo == 128 * j - 512 * i:  # block starts on the diagonal
                    nc.vector.tensor_mul(
                        pt[:, :, lo:lo + 128],
                        pt[:, :, lo:lo + 128],
                        mask_sb[:, None, :].to_broadcast([128, 2, 128]),
                    )
                return pt

            def emit_av(i, p, j, nj, pt):
                lo = max(0, 128 * j - 512 * i)
                if j == 0:
                    ctx_ps[(p, 0)] = cdp.tile([65, 512], F32, tag="ctxA",
                                              bufs=1, name=f"cA_{i}_{p}")
                    ctx_ps[(p, 1)] = cdp.tile([65, 512], F32, tag="ctxB",
                                              bufs=1, name=f"cB_{i}_{p}")
                nc.tensor.matmul(
                    ctx_ps[(p, 0)][:, lo:512], vt[j][:, 2 * p, :],
                    pt[:, 0, lo:512],
                    start=(j == 0), stop=(j == nj - 1),
                )
                nc.tensor.matmul(
                    ctx_ps[(p, 1)][:, lo:512], vt[j][:, 2 * p + 1, :],
                    pt[:, 1, lo:512],
                    start=(j == 0), stop=(j == nj - 1),
                )

            def emit_stash(i, p, den_g):
                """After last AV of (i, p): evict ctx (bf16; row 64 is the
                denominator), gather den rows into den_g (sync queue)."""
                tsl = slice(512 * i, 512 * (i + 1))
                cA, cB = ctx_ps[(p, 0)], ctx_ps[(p, 1)]
                # even head (rows 0-64 incl den row) into qt dead columns;
                # row 64 clobbers odd head's first Q row - already consumed
                nc.vector.tensor_copy(qt[p][0:65, tsl], cA[:, :])
                stashB = normp.tile([65, 512], BF16, tag="stB", bufs=2,
                                    name=f"stB_{i}_{p}")
                nc.vector.tensor_copy(stashB[:, :], cB[:, :])
                # gather den rows into den_g rows 2p / 2p+1  (SBUF->SBUF)
                nc.sync.dma_start(den_g[2 * p:2 * p + 1, :],
                                  qt[p][64:65, tsl])
                nc.sync.dma_start(den_g[2 * p + 1:2 * p + 2, :],
                                  stashB[64:65, :])
                # odd head: repack into qt rows 64-127 (partition move)
                nc.sync.dma_start(qt[p][64:128, tsl], stashB[0:64, :])

            def emit_norm_fillers(i, den_g):
                """Per-row normalization as 5 filler closures (interleaved
                into the next row's emission so the PE never waits on it)."""
                tsl = slice(512 * i, 512 * (i + 1))
                rec = normp.tile([8, 512], F32, tag="rec", bufs=2,
                                 name=f"rec_{i}")
                den_f = normp.tile([8, 512], F32, tag="denf", bufs=2,
                                   name=f"denf_{i}")
                rec_r = normp.tile([8, 512], F32R, tag="recr", bufs=2,
                                   name=f"recr_{i}")

                def recip():
                    nc.vector.tensor_copy(den_f[:], den_g[:])
                    nc.vector.reciprocal_approx_fast(rec[:], den_f[:])
                    nc.vector.tensor_copy(rec_r[:], rec[:])

                def norm_pair(p):
                    bc = stp.tile([128, 512], F32, tag="st", bufs=2,
                                  name=f"bc_{i}_{p}")
                    nc.tensor.matmul(
                        bc[:], onehot_sb[:, p * 128:(p + 1) * 128],
                        rec_r[:])
                    bc_sb = normp.tile([128, 512], F32, tag="bcsb", bufs=2,
                                       name=f"bcsb_{i}_{p}")
                    nc.vector.tensor_copy(bc_sb[:], bc[:])
                    nc.vector.tensor_mul(
                        qt[p][:, tsl], qt[p][:, tsl], bc_sb[:])

                return [recip] + [
                    (lambda p_: (lambda: norm_pair(p_)))(p) for p in range(NPAIR)]

            # ---------------- out projection ----------------
            pso_pool = [None]  # opened after qkv_pool releases

            def emit_proj_chunk(i, ec):
                tsl = slice(i * 512, (i + 1) * 512)
                ecs = slice(ec * 128, (ec + 1) * 128)
                pso = pso_pool[0].tile([128, 512], F32, tag="pso", bufs=2,
                                       name=f"pso_{i}_{ec}")
                for p in range(NPAIR):
                    nc.tensor.matmul(
                        pso[:], wo_sb[:, p, ecs], qt[p][:, tsl],
                        start=(p == 0), stop=(p == NPAIR - 1),
                    )
                ot = projp.tile([128, 512], F32, tag="ot", bufs=4,
                                name=f"ot_{i}_{ec}")
                nc.vector.tensor_copy(ot[:], pso[:])
                nc.sync.dma_start(out_d[ecs, tsl], ot[:])

            # ---------------- emission schedule ----------------
            den_gs = {}

            def emit_attn_row(i, fillers):
                """Emit attention row i with `fillers` (list of closures)
                interleaved evenly between units.  The AV matmuls trail the
                score matmuls by one unit (software pipeline) so the
                in-order PE never waits on the exp of the unit it just
                scored."""
                nj = 4 * i + 4
                units = [(p, j) for p in range(NPAIR) for j in range(nj)]
                den_gs[i] = normp.tile([8, 512], BF16, tag="deng", bufs=2,
                                       name=f"deng_{i}")
                nf, nu = len(fillers), len(units)
                fi = 0
                pending = None
                for u, (p, j) in enumerate(units):
                    while fi < nf and fi * nu <= u * nf:
                        fillers[fi]()
                        fi += 1
                    pt = emit_scores(i, p, j)
                    if pending is not None:
                        emit_av(*pending)
                        if pending[2] == nj - 1:  # last j of previous pair
                            emit_stash(i, pending[1], den_gs[i])
                    pending = (i, p, j, nj, pt)
                emit_av(*pending)
                emit_stash(i, NPAIR - 1, den_gs[i])
                while fi < nf:
                    fillers[fi]()
                    fi += 1
                return emit_norm_fillers(i, den_gs[i])

            # stage 0
            for c in range(12):
                emit_qkv_chain(0, c)
            # stages 1..3: QKV(s) interleaved with attention row s-1;
            # row s-2's normalization rides in the same filler stream
            norm_f = []
            for s_ in range(1, NT):
                chains = [(lambda ts, c: (lambda: emit_qkv_chain(ts, c)))(s_, c)
                          for c in range(12)]
                norm_f = emit_attn_row(s_ - 1, norm_f + chains)
            # qkv accum banks -> out-projection accum banks
            qkv_pool.release()
            pso_pool[0] = tc.alloc_tile_pool(name="psop", bufs=1, space="PSUM")
            # stage 4: attention row 3 interleaved with out-proj rows 0..1;
            # proj row 2 is held back so the PE has work to chew on while
            # row 3's norm chain (gather -> recip -> bc) resolves
            projs = [(lambda i, ec: (lambda: emit_proj_chunk(i, ec)))(i, ec)
                     for i in range(NT - 2) for ec in range(ND)]
            norm_f = emit_attn_row(NT - 1, norm_f + projs)
            for ec in range(ND):
                emit_proj_chunk(NT - 2, ec)
            for f in norm_f:
                f()
            for ec in range(ND):
                emit_proj_chunk(NT - 1, ec)
            pso_pool[0].release()

    nc.compile()
    return nc


def _get_program():
    global _PROGRAM
    if _PROGRAM is None:
        _PROGRAM = _build_program()
    return _PROGRAM


def _bf16(a):
    return np.ascontiguousarray(a.astype(ml_dtypes.bfloat16))


def _prep_in_maps(in_features, token_positions, W_qkv, W_out):
    in_features = np.asarray(in_features, dtype=np.float32)
    token_positions = np.asarray(token_positions)
    W_qkv = np.asarray(W_qkv, dtype=np.float32)
    W_out = np.asarray(W_out, dtype=np.float32)

    # RoPE pair permutation: [x0 of freq 0..31 | x1 of freq 0..31]
    perm = np.concatenate([np.arange(0, DH, 2), np.arange(1, DH, 2)])

    wqkv, wo = [], []
    for tp in range(TP):
        rows = []
        for sect in range(2):  # Q, K (permuted)
            for h in range(HLOC):
                g = tp * HLOC + h
                rows.append(W_qkv[sect * D + g * DH + perm])
        for h in range(HLOC):  # V natural
            g = tp * HLOC + h
            rows.append(W_qkv[2 * D + g * DH:2 * D + (g + 1) * DH])
        Wl = np.concatenate(rows, axis=0)      # [1536, 1024]
        wqkv.append(_bf16(Wl.T.reshape(ND, 128, 3 * HLOC * DH)))
        # wo[r, p, e] = W_out[e, (tp*HLOC + 2p + r//64)*DH + r%64]
        w = np.stack([
            np.concatenate([
                W_out[:, (tp * HLOC + 2 * p) * DH:(tp * HLOC + 2 * p + 1) * DH].T,
                W_out[:, (tp * HLOC + 2 * p + 1) * DH:(tp * HLOC + 2 * p + 2) * DH].T,
            ], axis=0) for p in range(NPAIR)
        ])                                     # [p, 128, 1024]
        wo.append(_bf16(w.transpose(1, 0, 2)))

    half = DH // 2
    inv_freq = (THETA ** (-2.0 * np.arange(half, dtype=np.float32) / DH)
                ).astype(np.float32)
    ang = token_positions.astype(np.float32)[:, None] * inv_freq[None, :]
    cos_t = np.cos(ang).T.astype(np.float32)   # [32, S]
    sin_t = np.sin(ang).T.astype(np.float32)
    cos128 = _bf16(np.tile(cos_t, (4, 1)))
    sin128 = _bf16(np.tile(np.concatenate([-sin_t, sin_t], axis=0), (2, 1)))
    # mask[kv, c] = 1 iff kv <= c (scores stored transposed: [kv, q])
    mask128 = _bf16(np.triu(np.ones((128, 128), dtype=np.float32)))
    # onehot[h, p*128 + c] = 1 iff h == 2p + c//64
    onehot = np.zeros((8, NPAIR * 128), dtype=np.float32)
    for p in range(NPAIR):
        onehot[2 * p, p * 128:p * 128 + 64] = 1.0
        onehot[2 * p + 1, p * 128 + 64:p * 128 + 128] = 1.0

    in_maps = []
    for c in range(NCORES):
        b, tp = c // 2, c % 2
        xh = _bf16(
            in_features[b].reshape(NT, 512, ND, 128).transpose(3, 0, 2, 1))
        in_maps.append({
            "x": xh,
            "wqkv": wqkv[tp],
            "wo": wo[tp],
            "cosT": cos128,
            "sinT": sin128,
            "mask": mask128,
            "onehot": onehot,
        })
    return in_maps


def run(in_features, token_positions, W_qkv, W_out, **spmd_kwargs):
    """Run the kernel; returns (output [B,S,D] f32, BassKernelResults)."""
    in_maps = _prep_in_maps(in_features, token_positions, W_qkv, W_out)
    nc = _get_program()
    res = run_bass_kernel_spmd(nc, in_maps, core_ids=list(range(NCORES)),
                               **spmd_kwargs)
    outs = [res.results[c]["out"] for c in range(NCORES)]
    full = np.stack([(outs[2 * b] + outs[2 * b + 1]).T for b in range(B)])
    return full.astype(np.float32), res


def kernel(in_features, token_positions, W_qkv, W_out):
    out, _ = run(in_features, token_positions, W_qkv, W_out)
    return out
